# revision 15
# baseline (speedup 1.0000x reference)
"""Trainium2 Bass kernel for the 1D advection stencil (slope-limited flux).

Math (axis=-1, L = N + 4 ghost cells, th = 2.0):
    flux = rho * v
    d[i]  = flux[i+1] - flux[i]
    hs[i] = minmod3(d[i], (d[i]+d[i+1])/4, d[i+1])        # == 0.5*minmod3(c0,c1,c2)
    p[i]  = flux[i+1] - hs[i];  q[i] = flux[i+1] + hs[i]
    pm[i] = (v[i+1] < 0) * p[i];  qm[i] = (v[i+1] > 0) * q[i]
    pm[L-3] = 0; qm[0] = 0
    fn[j]  = pm[j+1] + qm[j]
    out[i] = fn[i] - fn[i+1]
minmod3(a,b,c) = max(min3, min(max3, 0)) -- selects min-|.| when all same
sign else 0.  The whole minmod fuses into ONE custom DVE op (8 ALU stages).

Sharding: pure data-parallel over the leading batch axis B=16 -> 2 slabs
per core on 8 cores.  No halo exchange needed (stencil couples only along
the last axis, which stays whole on every core).
"""

import numpy as np

import concourse.bass as bass
import concourse.mybir as mybir
from concourse.mybir import AluOpType
from concourse.tile import TileContext
from concourse.bass_utils import run_bass_kernel_spmd

# Problem shape (hardcoded; kernel.py must be self-contained).
B, M, L = 16, 256, 8192
NCORES = 8
BP = B // NCORES            # 2 batch slabs per core
ROWS = BP * M               # 512 rows per core
RT = ROWS // 128            # 4 partition tiles of 128 rows
OUT_L = L - 4               # 8188
CHUNK = 2048                # output columns per inner tile
F32 = mybir.dt.float32

USE_CUSTOM_MINMOD = False

_MINMOD_OP = None


def _register_minmod():
    """Register the fused minmod3 custom DVE op (8 ALU stages, 2 streams).

    hs = max(min(min(a,b), s4), min(max(a,b), s4) min 0) with s4=(a+b)*s0.
    """
    global _MINMOD_OP
    if _MINMOD_OP is not None:
        return _MINMOD_OP
    import concourse.dve_ops as D
    from concourse.dve_spec import Spec, Src0, Src1, Zero, minn, maxx, lower
    from concourse.dve_uop import DveOpSpec

    name = "ANT_ADVECT_MINMOD"
    for op in D.OPS:
        if op.name == name:
            _MINMOD_OP = op
            return op

    a, b = Src0, Src1
    from concourse.dve_spec import C0
    s4 = (a + b) * C0
    body = maxx(minn(minn(a, b), s4), minn(maxx(maxx(a, b), s4), Zero))

    def _ref(in0, in1, s0, s1, imm2):
        s4v = (in0 + in1) * s0
        lo = np.minimum(np.minimum(in0, in1), s4v)
        hi = np.maximum(np.maximum(in0, in1), s4v)
        return np.maximum(lo, np.minimum(hi, 0.0)).astype(np.float32)

    spec = Spec(body=body, reference=_ref)
    row = D._CUSTOM_DVE_ROW_BASE + len(D.OPS)
    shas = {}
    for ver in ("v3", "v4"):
        try:
            compiled = DveOpSpec(
                name=name, opcode=row, uops=lower(spec, ver=ver), rd1_en=True
            )
            shas[ver] = compiled.sha(ver)
        except Exception:
            pass
    op = D.DveOp(name, spec, subdim=False, uops_sha=shas)
    D.OPS.append(op)
    D.CUSTOM_DVE_SPECS[name] = spec
    D._SUB_OPCODE_FOR_NAME[name] = row
    _MINMOD_OP = op
    return op


def _emit_minmod_stock(nc, wk, hs, d, C, POOL=None):
    if POOL is None:
        POOL = nc.gpsimd
    """hs = minmod3(d0, (d0+d1)/4, d1) in 6 stock ops.

    u = min(d0,d1)         (Pool)
    s = d0+d1              (Pool)
    w = max(d0,d1)         (DVE)
    lo = min(u, 0.25*s)    (DVE STT)
    hi = max(w, 0.25*s)    (DVE STT)
    hs = max(lo, min(hi,0))(DVE STT)
    """
    u = wk.tile([128, C + 2], F32, tag="mm_u")
    nc.vector.tensor_tensor(u[:], d[:, 0:C + 2], d[:, 1:C + 3], AluOpType.min)
    s = wk.tile([128, C + 2], F32, tag="mm_s")
    POOL.tensor_tensor(s[:], d[:, 0:C + 2], d[:, 1:C + 3], AluOpType.add)
    w = wk.tile([128, C + 2], F32, tag="mm_w")
    nc.vector.tensor_tensor(w[:], d[:, 0:C + 2], d[:, 1:C + 3], AluOpType.max)
    lo = wk.tile([128, C + 2], F32, tag="mm_lo")
    nc.vector.scalar_tensor_tensor(
        lo[:], s[:], 0.25, u[:], AluOpType.mult, AluOpType.min
    )
    hi = wk.tile([128, C + 2], F32, tag="mm_hi")
    nc.vector.scalar_tensor_tensor(
        hi[:], s[:], 0.25, w[:], AluOpType.mult, AluOpType.max
    )
    nc.vector.scalar_tensor_tensor(
        hs[:], hi[:], 0.0, lo[:], AluOpType.min, AluOpType.max
    )


def _split_multi_waits(nc):
    """Walrus in this environment rejects instructions carrying more than
    one sync wait ("Too many sync wait commands").  Tile freely attaches
    several.  Split: for an instruction with k>1 waits, emit k-1 engine
    NoOps (one wait each) immediately before it, leaving one wait on the
    instruction itself."""
    import copy
    import concourse.mybir as mybir

    counter = [0]

    def mk_nop(engine, wait):
        counter[0] += 1
        return mybir.InstNoOp(
            name=f"waitsplit-{counter[0]}",
            engine=engine,
            ins=[],
            outs=[],
            sync_info=mybir.SyncInfo(on_wait=[wait], on_update=[]),
        )

    m = nc.m
    new_module = copy.replace(m, functions=[])
    for function in m.functions:
        new_function = copy.replace(function, blocks=[])
        new_function.set_allocations_from_list(function.allocations)
        for block in function.blocks:
            new_insts = []
            for inst in block.instructions:
                si = inst.sync_info
                waits = list(si.on_wait) if (si and si.on_wait) else []
                if len(waits) > 1:
                    for w in waits[:-1]:
                        new_insts.append(mk_nop(inst.engine, w))
                    inst.sync_info = mybir.SyncInfo(
                        on_wait=[waits[-1]], on_update=list(si.on_update)
                    )
                new_insts.append(inst)
            new_function.blocks.append(
                copy.replace(block, instructions=new_insts)
            )
        new_module.functions.append(new_function)
    nc.m = new_module


def build_module(repeat=1, variant="dve"):
    """repeat>1 wraps the whole body in a device-side For_i loop --
    benchmark-only (re-reads the same inputs, re-writes the same outputs)
    so device time dominates the axon tunnel overhead.

    variant: "split" (DVE+Pool), "dve" (all compute on DVE),
             "dma" (transfers only -- roofline probe)."""
    import contextlib
    nc = bass.Bass()
    rho = nc.dram_tensor("rho", [ROWS, L], F32, kind="ExternalInput")
    vin = nc.dram_tensor("v", [ROWS, L], F32, kind="ExternalInput")
    out = nc.dram_tensor("out", [ROWS, OUT_L], F32, kind="ExternalOutput")

    mm_op = _register_minmod() if USE_CUSTOM_MINMOD else None

    with TileContext(nc) as tc:
        with (
            tc.tile_pool(name="io", bufs=3) as io,
            tc.tile_pool(name="wk", bufs=1) as wk,
            (tc.For_i(0, repeat, 1) if repeat > 1 else contextlib.nullcontext()),
        ):
            POOL = nc.gpsimd if variant == "split" else nc.vector
            TAILPOOL = nc.gpsimd if variant in ("split", "split2") else nc.vector
            for rt in range(RT):
                r0 = rt * 128
                c0 = 0
                while c0 < OUT_L:
                    C = min(CHUNK, OUT_L - c0)
                    S = C + 4
                    rho_t = io.tile([128, S], F32, tag="rho")
                    nc.sync.dma_start(rho_t[:], rho[r0:r0 + 128, c0:c0 + S])
                    v_t = io.tile([128, S], F32, tag="v")
                    nc.sync.dma_start(v_t[:], vin[r0:r0 + 128, c0:c0 + S])
                    if variant == "dma":
                        out_t = io.tile([128, C], F32, tag="out")
                        nc.vector.tensor_tensor(
                            out_t[:, 0:1], rho_t[:, 0:1], v_t[:, 0:1],
                            AluOpType.mult,
                        )
                        nc.sync.dma_start(
                            out[r0:r0 + 128, c0:c0 + C], out_t[:]
                        )
                        c0 += C
                        continue

                    # 1. flux = rho * v           (DVE)
                    flux = wk.tile([128, S], F32, tag="flux")
                    nc.vector.tensor_tensor(
                        flux[:], rho_t[:], v_t[:], AluOpType.mult
                    )
                    # 2. d = flux[1:] - flux[:-1] (Pool)
                    d = wk.tile([128, S - 1], F32, tag="d")
                    POOL.tensor_tensor(
                        d[:], flux[:, 1:S], flux[:, 0:S - 1], AluOpType.subtract
                    )
                    # 3. hs = minmod3(d0, (d0+d1)/4, d1)  (DVE, fused)
                    hs = wk.tile([128, C + 2], F32, tag="hs")
                    if mm_op is not None:
                        nc.vector._custom_dve(
                            mm_op, out=hs[:], in0=d[:, 0:C + 2],
                            in1=d[:, 1:C + 3], s0=0.25,
                        )
                    else:
                        _emit_minmod_stock(nc, wk, hs, d, C, POOL)
                    # 4./5. p = f1 - hs ; q = f1 + hs   (DVE)
                    p = wk.tile([128, C + 2], F32, tag="p")
                    nc.vector.tensor_tensor(
                        p[:], flux[:, 1:C + 3], hs[:], AluOpType.subtract
                    )
                    q = wk.tile([128, C + 2], F32, tag="q")
                    nc.vector.tensor_tensor(
                        q[:], flux[:, 1:C + 3], hs[:], AluOpType.add
                    )
                    # 6./7. pm = (v1<0)*p ; qm = (v1>0)*q   (DVE, fused cmp+mul)
                    pm = wk.tile([128, C + 2], F32, tag="pm")
                    nc.vector.scalar_tensor_tensor(
                        pm[:], v_t[:, 1:C + 3], 0.0, p[:],
                        AluOpType.is_lt, AluOpType.mult,
                    )
                    qm = wk.tile([128, C + 2], F32, tag="qm")
                    nc.vector.scalar_tensor_tensor(
                        qm[:], v_t[:, 1:C + 3], 0.0, q[:],
                        AluOpType.is_gt, AluOpType.mult,
                    )
                    # global boundary conditions
                    if c0 == 0:
                        nc.vector.memset(qm[:, 0:1], 0.0)
                    if c0 + C == OUT_L:
                        nc.vector.memset(pm[:, C + 1:C + 2], 0.0)
                    # 8. fn = pm[1:] + qm[:-1]   (Pool)
                    fn = wk.tile([128, C + 1], F32, tag="fn")
                    TAILPOOL.tensor_tensor(
                        fn[:], pm[:, 1:C + 2], qm[:, 0:C + 1], AluOpType.add
                    )
                    # 9. out = fn[:-1] - fn[1:]  (Pool)
                    out_t = io.tile([128, C], F32, tag="out")
                    TAILPOOL.tensor_tensor(
                        out_t[:], fn[:, 0:C], fn[:, 1:C + 1], AluOpType.subtract
                    )
                    nc.sync.dma_start(out[r0:r0 + 128, c0:c0 + C], out_t[:])
                    c0 += C
    _split_multi_waits(nc)
    return nc


_NC_CACHE = None


def _get_nc():
    global _NC_CACHE
    if _NC_CACHE is None:
        _NC_CACHE = build_module()
    return _NC_CACHE


def kernel(rho, v, axis=2, retain_padding=0, **_kw):
    rho = np.ascontiguousarray(np.asarray(rho, dtype=np.float32))
    v = np.ascontiguousarray(np.asarray(v, dtype=np.float32))
    assert rho.shape == (B, M, L) and v.shape == (B, M, L)

    nc = _get_nc()
    in_maps = [
        {
            "rho": rho[c * BP:(c + 1) * BP].reshape(ROWS, L),
            "v": v[c * BP:(c + 1) * BP].reshape(ROWS, L),
        }
        for c in range(NCORES)
    ]
    last_err = None
    for _attempt in range(3):
        try:
            res = run_bass_kernel_spmd(
                nc, in_maps, core_ids=list(range(NCORES))
            )
            break
        except Exception as e:  # rare transient NRT device errors
            last_err = e
            import time as _time
            _time.sleep(5)
    else:
        raise last_err
    outs = [r["out"].reshape(BP, M, OUT_L) for r in res.results]
    return np.concatenate(outs, axis=0)


def run_traced(rho, v):
    """Like kernel() but with trace=True; returns (out, BassKernelResults)."""
    rho = np.ascontiguousarray(np.asarray(rho, dtype=np.float32))
    v = np.ascontiguousarray(np.asarray(v, dtype=np.float32))
    nc = _get_nc()
    in_maps = [
        {
            "rho": rho[c * BP:(c + 1) * BP].reshape(ROWS, L),
            "v": v[c * BP:(c + 1) * BP].reshape(ROWS, L),
        }
        for c in range(NCORES)
    ]
    res = run_bass_kernel_spmd(
        nc, in_maps, core_ids=list(range(NCORES)), trace=True
    )
    outs = [r["out"].reshape(BP, M, OUT_L) for r in res.results]
    return np.concatenate(outs, axis=0), res
